# revision 3
# baseline (speedup 1.0000x reference)
"""HGNNConv2 Bass kernel for Trainium2, 8 NeuronCores.

Computation (reference):
    Xt = X @ W.T + b
    dv = rowdeg(H)^-1/2 ; de = coldeg(H)^-1
    Y  = relu(dv * (H @ (de * (H.T @ (dv * Xt)))))

Sharding: rows (vertices) of X and H split 8 ways. Each core computes
edge partials E_c = Hs.T @ (dv*Xt_s) plus column-degree partials, one
AllReduce sums them, then Y_s = dv * (Hs @ (de * E)) is local per shard.

H is a 0/1 incidence matrix, so it is shipped to the device losslessly
as fp8e4m3 in both layouts ([v,m] for pass 1, [m,v] for pass 2) - this
halves HBM traffic twice over f32 and avoids any on-chip transpose.
Matmuls use mixed fp8 (H) x bf16 (dense operand); accumulation is fp32.
"""

import os
import sys

sys.path.insert(0, "/opt/trn_rl_repo")

import numpy as np
import ml_dtypes

N, M, CI, CO = 16384, 8192, 128, 64
NCORES = 8
NS = N // NCORES            # 2048 vertex rows per core
VB = NS // 128              # 16 vertex blocks of 128
MB = M // 128               # 64 edge panels of 128

_built_nc = None
LAST_RESULTS = None


def _build():
    from concourse import bacc, tile, mybir

    f32 = mybir.dt.float32
    bf16 = mybir.dt.bfloat16
    f8 = mybir.dt.float8e4
    AX = mybir.AxisListType.X
    OP = mybir.AluOpType
    AF = mybir.ActivationFunctionType

    nc = bacc.Bacc("TRN2", target_bir_lowering=False, debug=False,
                   num_devices=NCORES)

    xst = nc.dram_tensor("xst", [CI, NS], bf16, kind="ExternalInput")
    hs = nc.dram_tensor("hs", [NS, M], f8, kind="ExternalInput")
    hst = nc.dram_tensor("hst", [M, NS], f8, kind="ExternalInput")
    wt = nc.dram_tensor("wt", [CI, CO], bf16, kind="ExternalInput")
    bb = nc.dram_tensor("bb", [128, CO], f32, kind="ExternalInput")
    y = nc.dram_tensor("y", [NS, CO], f32, kind="ExternalOutput")

    with tile.TileContext(nc) as tc:
        with (
            tc.tile_pool(name="persist", bufs=1) as persist,
            tc.tile_pool(name="hpool", bufs=3) as hpool,
            tc.tile_pool(name="zpool", bufs=3) as zpool,
            tc.tile_pool(name="spool", bufs=8) as spool,
            tc.tile_pool(name="stpool", bufs=2) as stpool,
            tc.tile_pool(name="dram", bufs=1, space="DRAM") as dram,
        ):
            # ---- constants / small inputs
            xst_sb = persist.tile([128, NS], bf16)
            nc.sync.dma_start(xst_sb[:], xst.ap())
            wt_sb = persist.tile([128, CO], bf16)
            nc.sync.dma_start(wt_sb[:], wt.ap())
            b_sb = persist.tile([128, CO], f32)
            nc.sync.dma_start(b_sb[:], bb.ap())

            # ---- resident transposed H (fp8) + column-degree partials
            hst_res = persist.tile([128, MB, NS], f8)     # 16 MB
            cs_sb = persist.tile([128, MB], f32)
            for g in range(8):
                src = hst.ap()[g * 1024:(g + 1) * 1024, :]
                nc.sync.dma_start(
                    hst_res[:, g * 8:(g + 1) * 8, :],
                    src.rearrange("(j p) v -> p j v", p=128),
                )
                for jj in range(8):
                    j = g * 8 + jj
                    nc.vector.tensor_reduce(
                        cs_sb[:, j:j + 1], hst_res[:, j, :], AX, OP.add)

            # ---- stage A: Xt = Xs @ W.T + b   (per 128-row block)
            xt_all = persist.tile([128, VB, CO], f32)
            with tc.tile_pool(name="psA", bufs=2, space="PSUM") as psA:
                for i in range(VB):
                    xt_ps = psA.tile([128, CO], f32)
                    nc.tensor.matmul(
                        xt_ps[:], xst_sb[:, i * 128:(i + 1) * 128], wt_sb[:],
                        start=True, stop=True)
                    nc.vector.tensor_tensor(
                        xt_all[:, i, :], xt_ps[:], b_sb[:], OP.add)

            # ---- pass 1: row degrees, Z = dv*Xt, E = Hs.T @ Z
            dv_all = persist.tile([128, VB], f32)
            e_sb = persist.tile([128, MB, CO], f32)
            with tc.tile_pool(name="psE", bufs=1, space="PSUM") as psE:
                e_ps = psE.tile([128, MB, CO], f32)       # all 8 banks
                for i in range(VB):
                    hs_i = hpool.tile([128, M], f8)
                    nc.sync.dma_start(hs_i[:], hs.ap()[i * 128:(i + 1) * 128, :])
                    rs = spool.tile([128, 1], f32)
                    nc.vector.tensor_reduce(rs[:], hs_i[:], AX, OP.add)
                    rc = spool.tile([128, 1], f32)
                    nc.vector.tensor_scalar_max(rc[:], rs[:], 0.5)
                    sq = spool.tile([128, 1], f32)
                    nc.scalar.sqrt(sq[:], rc[:])
                    rq = spool.tile([128, 1], f32)
                    nc.vector.reciprocal(rq[:], sq[:])
                    mk = spool.tile([128, 1], f32)
                    nc.vector.tensor_scalar_min(mk[:], rs[:], 1.0)
                    nc.vector.tensor_tensor(
                        dv_all[:, i:i + 1], rq[:], mk[:], OP.mult)
                    z_i = zpool.tile([128, CO], bf16)
                    nc.vector.tensor_scalar(
                        z_i[:], xt_all[:, i, :], dv_all[:, i:i + 1], None,
                        OP.mult)
                    for j in range(MB):
                        # start=True clears has_written for the WHOLE bank;
                        # issue it only on the first matmul touching a bank.
                        nc.tensor.matmul(
                            e_ps[:, j, :], hs_i[:, j * 128:(j + 1) * 128],
                            z_i[:], start=(i == 0 and j % 8 == 0),
                            stop=(i == VB - 1), skip_group_check=True)
                nc.scalar.copy(e_sb[:], e_ps[:])

            # ---- AllReduce E partials + column-degree partials
            bounce_in = dram.tile([128, MB + 1, CO], f32)
            bounce_out = dram.tile([128, MB + 1, CO], f32, addr_space="Shared")
            nc.sync.dma_start(bounce_in[:, :MB, :], e_sb[:])
            nc.sync.dma_start(bounce_in[:, MB, :], cs_sb[:])
            nc.gpsimd.collective_compute(
                "AllReduce", OP.add,
                replica_groups=[list(range(NCORES))],
                ins=[bounce_in.opt()], outs=[bounce_out.opt()])

            # ---- de = 1/coldeg ; Ye = de * E
            cssum = spool.tile([128, MB], f32)
            nc.sync.dma_start(cssum[:], bounce_out[:, MB, :])
            ccl = spool.tile([128, MB], f32)
            nc.vector.tensor_scalar_max(ccl[:], cssum[:], 0.5)
            rr = spool.tile([128, MB], f32)
            nc.vector.reciprocal(rr[:], ccl[:])
            mk2 = spool.tile([128, MB], f32)
            nc.vector.tensor_scalar_min(mk2[:], cssum[:], 1.0)
            de_sb = persist.tile([128, MB], f32)
            nc.vector.tensor_tensor(de_sb[:], rr[:], mk2[:], OP.mult)

            ye_sb = persist.tile([128, MB, CO], bf16)
            for k in range(8):
                st = stpool.tile([128, 8, CO], f32)
                nc.sync.dma_start(st[:], bounce_out[:, k * 8:(k + 1) * 8, :])
                for jj in range(8):
                    j = k * 8 + jj
                    nc.vector.tensor_scalar(
                        ye_sb[:, j, :], st[:, jj, :], de_sb[:, j:j + 1], None,
                        OP.mult)

            # ---- pass 2: Y = relu(dv * (Hs @ Ye))
            y_sb = persist.tile([128, VB, CO], f32)
            with tc.tile_pool(name="psY", bufs=1, space="PSUM") as psY:
                y_ps = psY.tile([128, VB, CO], f32)       # 2 banks
                for j in range(MB):
                    for i in range(VB):
                        nc.tensor.matmul(
                            y_ps[:, i, :],
                            hst_res[:, j, i * 128:(i + 1) * 128],
                            ye_sb[:, j, :],
                            start=(j == 0 and i % 8 == 0),
                            stop=(j == MB - 1), skip_group_check=True)
                for i in range(VB):
                    nc.scalar.activation(
                        y_sb[:, i, :], y_ps[:, i, :], AF.Relu,
                        scale=dv_all[:, i:i + 1])
            nc.sync.dma_start(
                y.ap().rearrange("(i p) c -> p i c", p=128), y_sb[:])

    nc.compile()
    return nc


def _get_nc():
    global _built_nc
    if _built_nc is None:
        _built_nc = _build()
    return _built_nc


def kernel(X, H, W, b):
    from concourse.bass_utils import run_bass_kernel_spmd

    global LAST_RESULTS
    nc = _get_nc()

    f8 = ml_dtypes.float8_e4m3
    bf16 = ml_dtypes.bfloat16
    X = np.asarray(X, dtype=np.float32)
    H = np.asarray(H, dtype=np.float32)
    W = np.asarray(W, dtype=np.float32)
    b = np.asarray(b, dtype=np.float32)

    wt_np = np.ascontiguousarray(W.T).astype(bf16)
    bb_np = np.ascontiguousarray(np.broadcast_to(b, (128, CO))).astype(np.float32)

    in_maps = []
    for c in range(NCORES):
        r0, r1 = c * NS, (c + 1) * NS
        in_maps.append({
            "xst": np.ascontiguousarray(X[r0:r1].T).astype(bf16),
            "hs": H[r0:r1].astype(f8),
            "hst": np.ascontiguousarray(H[r0:r1].T).astype(f8),
            "wt": wt_np,
            "bb": bb_np,
        })

    trace = bool(int(os.environ.get("HGNN_TRACE", "0")))
    res = run_bass_kernel_spmd(nc, in_maps, list(range(NCORES)), trace=trace)
    LAST_RESULTS = res
    return np.concatenate([res.results[c]["y"] for c in range(NCORES)], axis=0)


if __name__ == "__main__":
    rng = np.random.default_rng(0)
    X = rng.standard_normal((N, CI), dtype=np.float32)
    H = (rng.random((N, M)) < 0.005).astype(np.float32)
    W = rng.standard_normal((CO, CI), dtype=np.float32) / np.sqrt(CI)
    b = (rng.standard_normal(CO) * 0.01).astype(np.float32)
    Y = kernel(X=X, H=H, W=W.astype(np.float32), b=b)
    print("out", Y.shape, Y.dtype, np.abs(Y).max())
